# revision 10
# baseline (speedup 1.0000x reference)
"""Causal multi-head attention block (B=4, S=2048, D=768, H=12, Dh=64)
distributed over 8 NeuronCores: core = (batch, head-group), each core
computes its 6 heads end-to-end plus its partial output projection;
host sums the two partials per batch and adds the bias.

Self-contained: hardcodes all shapes; no sibling imports.
"""

import numpy as np

B, S, D = 4, 2048, 768
H, DH = 12, 64
G = 384          # channels per head group (6 heads)
NPAIR = 3        # head pairs per core
NSC = 4          # 512-wide query windows
W = 512
NST = 16         # 128-row s-tiles
NDC = 6          # 128-row D chunks

_PROGRAM = None
PROFILE = False
PROFILE_DIR = None
LAST_RESULT = None


def _split_waits(nc, max_waits=1, max_updates=1):
    """This container's walrus rejects instructions carrying more than one
    semaphore wait/update ("Too many sync wait commands").  Move excess
    waits onto NoOps inserted before the owning instruction (same engine)
    and excess updates onto NoOps inserted after."""
    import concourse.mybir as mybir

    counter = [0]

    def nop(engine, waits, updates):
        counter[0] += 1
        n = mybir.InstNoOp(name=f"wsplit_nop_{counter[0]}", ins=[], outs=[])
        n.engine = engine
        n.sync_info = mybir.SyncInfo(on_wait=waits, on_update=updates)
        return n

    for bb in nc.main_func.blocks:
        out = []
        changed = False
        for ins in bb.instructions:
            si = ins.sync_info
            waits = list(si.on_wait) if si and si.on_wait else []
            updates = list(si.on_update) if si and si.on_update else []
            pre, post = [], []
            if len(waits) > max_waits:
                keep = waits[:max_waits - 1] if max_waits > 1 else []
                rest = waits[len(keep):]
                while rest:
                    chunk, rest = rest[:max_waits], rest[max_waits:]
                    pre.append(chunk)
                waits = keep
                changed = True
            if len(updates) > max_updates:
                rest = updates[max_updates:]
                updates = updates[:max_updates]
                while rest:
                    chunk, rest = rest[:max_updates], rest[max_updates:]
                    post.append(chunk)
                changed = True
            if pre or post:
                ins.sync_info = mybir.SyncInfo(
                    on_wait=waits, on_update=updates)
            for w in pre:
                out.append(nop(ins.engine, w, []))
            out.append(ins)
            for u in post:
                out.append(nop(ins.engine, [], u))
        if changed:
            bb.instructions = out


def _install_profile_hooks():
    """Dev-only (PROFILE=True): register the NTFF profile hook that the
    agent image's antenv lacks, and stub out the artifact upload."""
    import sys
    import types

    try:
        from antenv.axon_hooks import get_axon_ntff_profile_hook  # noqa: F401
    except ImportError:
        import antenv
        from trn_agent_boot import trn_boot

        hook = trn_boot._ntff_profile_via_ctypes("/opt/axon/libaxon_pjrt.so")
        mod = types.ModuleType("antenv.axon_hooks")
        mod._hook = hook
        mod.get_axon_ntff_profile_hook = lambda: mod._hook
        mod.set_axon_ntff_profile_hook = lambda h: setattr(mod, "_hook", h)
        sys.modules["antenv.axon_hooks"] = mod
        antenv.axon_hooks = mod

    from concourse import bass_utils

    bass_utils.upload_artifacts = lambda tmpdir: "local://" + tmpdir


def _build_program():
    import concourse.bass as bass
    import concourse.mybir as mybir
    import concourse.tile as tile

    f16 = mybir.dt.float16
    f32 = mybir.dt.float32

    nc = bass.Bass()
    xt_d = nc.declare_dram_parameter("xt", [D, S], f16, isOutput=False)
    wq_d = nc.declare_dram_parameter("wq", [D, G], f16, isOutput=False)
    wk_d = nc.declare_dram_parameter("wk", [D, G], f16, isOutput=False)
    wv_d = nc.declare_dram_parameter("wv", [D, G], f16, isOutput=False)
    wo_d = nc.declare_dram_parameter("wo", [G, D], f16, isOutput=False)
    mk_d = nc.declare_dram_parameter("mk", [128, 128], f16, isOutput=False)
    id_d = nc.declare_dram_parameter("idn", [128, 128], f16, isOutput=False)
    y_d = nc.declare_dram_parameter("y", [S, D], f32, isOutput=True)

    with tile.TileContext(nc) as tc:
        with (
            tc.tile_pool(name="const", bufs=1) as const,
            tc.tile_pool(name="work", bufs=3) as work,
            tc.tile_pool(name="outp", bufs=3) as outp,
            tc.tile_pool(name="ps", bufs=2, space="PSUM") as ps,
        ):
            # ---- persistent SBUF tiles ----
            xt = [const.tile([128, S], f16, name=f"xt{i}", tag=f"xt{i}")
                  for i in range(NDC)]
            wq = [const.tile([128, G], f16, name=f"wq{i}", tag=f"wq{i}")
                  for i in range(NDC)]
            wk = [const.tile([128, G], f16, name=f"wk{i}", tag=f"wk{i}")
                  for i in range(NDC)]
            wv = [const.tile([128, G], f16, name=f"wv{i}", tag=f"wv{i}")
                  for i in range(NDC)]
            wo = [const.tile([128, D], f16, name=f"wo{i}", tag=f"wo{i}")
                  for i in range(3)]
            qt = [const.tile([128, S], f16, name=f"qt{p}", tag=f"qt{p}")
                  for p in range(NPAIR)]
            kt = [const.tile([128, S], f16, name=f"kt{p}", tag=f"kt{p}")
                  for p in range(NPAIR)]
            vt = [const.tile([128, G], f16, name=f"vt{t}", tag=f"vt{t}")
                  for t in range(NST)]
            gt = [const.tile([128, S], f16, name=f"gt{p}", tag=f"gt{p}")
                  for p in range(NPAIR)]
            mk = const.tile([128, 128], f16, name="mk", tag="mk")
            idn = const.tile([128, 128], f16, name="idn", tag="idn")
            ones = const.tile([128, DH], f16, name="ones", tag="ones")

            # ---- input DMAs ----
            for i in range(NDC):
                nc.sync.dma_start(out=xt[i], in_=xt_d[128 * i:128 * (i + 1), :])
            for i in range(NDC):
                nc.sync.dma_start(out=wq[i], in_=wq_d[128 * i:128 * (i + 1), :])
                nc.sync.dma_start(out=wk[i], in_=wk_d[128 * i:128 * (i + 1), :])
                nc.sync.dma_start(out=wv[i], in_=wv_d[128 * i:128 * (i + 1), :])
            for i in range(3):
                nc.sync.dma_start(out=wo[i], in_=wo_d[128 * i:128 * (i + 1), :])
            nc.sync.dma_start(out=mk, in_=mk_d[:, :])
            nc.sync.dma_start(out=idn, in_=id_d[:, :])
            nc.vector.memset(ones, 1.0)

            # Pre-clear the two scores slots so diagonal-group exp calls that
            # sweep across the unwritten gap read defined values.
            for i in range(2):
                z = ps.tile([128, 1024], f32, name=f"z{i}", tag="sc", bufs=2)
                nc.vector.memset(z, 0.0)

            def proj_qk_unit(pair, sc):
                qp = ps.tile([128, W], f32, name=f"qp{pair}_{sc}",
                             tag="fa", bufs=1)
                for dc in range(NDC):
                    nc.tensor.matmul(
                        qp,
                        wq[dc][:, 128 * pair:128 * (pair + 1)],
                        xt[dc][:, W * sc:W * (sc + 1)],
                        start=(dc == 0), stop=(dc == NDC - 1))
                nc.vector.tensor_copy(
                    out=qt[pair][:, W * sc:W * (sc + 1)], in_=qp)
                kp = ps.tile([128, W], f32, name=f"kp{pair}_{sc}",
                             tag="fb", bufs=1)
                for dc in range(NDC):
                    nc.tensor.matmul(
                        kp,
                        wk[dc][:, 128 * pair:128 * (pair + 1)],
                        xt[dc][:, W * sc:W * (sc + 1)],
                        start=(dc == 0), stop=(dc == NDC - 1))
                nc.vector.tensor_copy(
                    out=kt[pair][:, W * sc:W * (sc + 1)], in_=kp)

            def proj_v(st):
                vp = ps.tile([128, G], f32, name=f"vp{st}", tag="fa", bufs=1)
                for dc in range(NDC):
                    nc.tensor.matmul(
                        vp,
                        xt[dc][:, 128 * st:128 * (st + 1)],
                        wv[dc],
                        start=(dc == 0), stop=(dc == NDC - 1))
                nc.vector.tensor_copy(out=vt[st], in_=vp)

            def outproj(st):
                o0 = ps.tile([128, G], f32, name=f"o0_{st}", tag="fa", bufs=1)
                for cc in range(3):
                    nc.tensor.matmul(
                        o0,
                        gt[cc][:, 128 * st:128 * (st + 1)],
                        wo[cc][:, 0:G],
                        start=(cc == 0), stop=(cc == 2))
                o1 = ps.tile([128, G], f32, name=f"o1_{st}", tag="fb", bufs=1)
                for cc in range(3):
                    nc.tensor.matmul(
                        o1,
                        gt[cc][:, 128 * st:128 * (st + 1)],
                        wo[cc][:, G:D],
                        start=(cc == 0), stop=(cc == 2))
                ob = outp.tile([128, D], f32, name=f"ob{st}", tag="ob", bufs=3)
                nc.vector.tensor_copy(out=ob[:, 0:G], in_=o0)
                nc.vector.tensor_copy(out=ob[:, G:D], in_=o1)
                nc.sync.dma_start(
                    out=y_d[128 * st:128 * (st + 1), :], in_=ob)

            def attention(pair, sc, filler_tick):
                jbs = list(range(4 * sc + 4))
                pv = ps.tile([128, W], f32, name=f"pv{pair}_{sc}",
                             tag="apv", bufs=1)
                dn = ps.tile([128, W], f32, name=f"dn{pair}_{sc}",
                             tag="adn", bufs=1)
                for jb in jbs:
                    col0 = max(0, 128 * jb - W * sc)
                    first, last = (jb == jbs[0]), (jb == jbs[-1])
                    diag = jb >= 4 * sc
                    sct = ps.tile([128, 1024], f32, name=f"sc{pair}_{sc}_{jb}",
                                  tag="sc", bufs=2)
                    # scores^T for the head pair (row-packed: K=64 each);
                    # on diagonal blocks accumulate the -inf causal mask via
                    # an identity matmul so ACT/DVE stay off the mask path
                    nc.tensor.matmul(
                        sct[:, col0:W],
                        kt[pair][0:64, 128 * jb:128 * (jb + 1)],
                        qt[pair][0:64, W * sc + col0:W * (sc + 1)],
                        start=True, stop=not diag)
                    if diag:
                        nc.tensor.matmul(
                            sct[:, col0:col0 + 128], idn, mk,
                            start=False, stop=True)
                    nc.tensor.matmul(
                        sct[:, W + col0:2 * W],
                        kt[pair][64:128, 128 * jb:128 * (jb + 1)],
                        qt[pair][64:128, W * sc + col0:W * (sc + 1)],
                        start=True, stop=not diag)
                    if diag:
                        nc.tensor.matmul(
                            sct[:, W + col0:W + col0 + 128], idn, mk,
                            start=False, stop=True)
                    ex = work.tile([128, 1024], f16, name=f"ex{pair}_{sc}_{jb}",
                                   tag="exp", bufs=4)
                    nc.scalar.activation(
                        out=ex[:, col0:1024], in_=sct[:, col0:1024],
                        func=mybir.ActivationFunctionType.Exp, scale=0.125)
                    # attn @ v (transposed out, col-packed pair) + denominators
                    nc.tensor.matmul(
                        pv[0:64, col0:W],
                        vt[jb][:, 128 * pair:128 * pair + 64],
                        ex[:, col0:W],
                        start=first, stop=last)
                    nc.tensor.matmul(
                        pv[64:128, col0:W],
                        vt[jb][:, 128 * pair + 64:128 * (pair + 1)],
                        ex[:, W + col0:2 * W],
                        start=first, stop=last)
                    nc.tensor.matmul(
                        dn[0:64, col0:W],
                        ones,
                        ex[:, col0:W],
                        start=first, stop=last)
                    nc.tensor.matmul(
                        dn[64:128, col0:W],
                        ones,
                        ex[:, W + col0:2 * W],
                        start=first, stop=last)
                    filler_tick()
                rc = work.tile([128, W], f32, name=f"rc{pair}_{sc}",
                               tag="rc", bufs=2)
                nc.vector.reciprocal(out=rc, in_=dn)
                nc.vector.tensor_mul(
                    gt[pair][:, W * sc:W * (sc + 1)], pv, rc)

            # ---- orchestration: keep PE fed during the ACT-bound
            # attention sections by interleaving projection/output work ----
            for sc in range(NSC):
                proj_qk_unit(0, sc)
            for st in range(4):
                proj_v(st)

            def run_pair(pair, fillers):
                # fillers: list of (emit_fn, earliest_sc, deadline_sc).
                # A unit may be emitted once the pair's window `earliest_sc`
                # has started (data it reads is ready), and MUST be emitted
                # before window `deadline_sc` starts (its output is consumed
                # there).  Deadline-constrained units must be listed first,
                # in ascending deadline order.
                total_groups = sum(4 * s + 4 for s in range(NSC))
                state = {"emitted": 0, "group": 0, "sc": 0}
                n_units = len(fillers)

                def tick():
                    state["group"] += 1
                    want = state["group"] * n_units // total_groups
                    while state["emitted"] < min(want, n_units):
                        fn, earliest, _dl = fillers[state["emitted"]]
                        if earliest is not None and earliest > state["sc"]:
                            break
                        fn()
                        state["emitted"] += 1

                for sc in range(NSC):
                    state["sc"] = sc
                    while state["emitted"] < n_units:
                        fn, _e, dl = fillers[state["emitted"]]
                        if dl is not None and dl <= sc:
                            fn()
                            state["emitted"] += 1
                        else:
                            break
                    attention(pair, sc, tick)
                while state["emitted"] < n_units:
                    fillers[state["emitted"]][0]()
                    state["emitted"] += 1

            # pair 0: remaining v tiles (v[st] needed from window st//4 on),
            # then pair-1 q/k projections
            f0 = [((lambda st=st: proj_v(st)), None, st // 4)
                  for st in range(4, NST)]
            f0 += [((lambda sc=sc: proj_qk_unit(1, sc)), None, None)
                   for sc in range(NSC)]
            run_pair(0, f0)
            # pair 1: pair-2 q/k projections
            f1 = [((lambda sc=sc: proj_qk_unit(2, sc)), None, None)
                  for sc in range(NSC)]
            run_pair(1, f1)
            # pair 2: output projection for windows already complete.
            # outproj(st) may only run after this pair's window st//4 is done
            f2 = [((lambda st=st: outproj(st)), (st // 4) + 1, None)
                  for st in range(12)]
            run_pair(2, f2)
            for st in range(12, NST):
                outproj(st)

    _split_waits(nc)
    return nc


def _get_program():
    global _PROGRAM
    if _PROGRAM is None:
        _PROGRAM = _build_program()
    return _PROGRAM


def kernel(x, Wq, Wk, Wv, Wo, bo):
    global LAST_RESULT
    from concourse.bass_utils import run_bass_kernel_spmd

    x = np.asarray(x, np.float32)
    Wq = np.asarray(Wq, np.float32)
    Wk = np.asarray(Wk, np.float32)
    Wv = np.asarray(Wv, np.float32)
    Wo = np.asarray(Wo, np.float32)
    bo = np.asarray(bo, np.float32)

    # additive causal mask in transposed layout: 0 where j<=i, -inf-ish else
    tri = np.tril(np.ones((128, 128), np.float32)).T  # 1 where j<=i
    mk = ((1.0 - tri) * -60000.0).astype(np.float16)
    idn = np.eye(128, dtype=np.float16)

    in_maps = []
    for c in range(8):
        b, g = divmod(c, 2)
        hs = slice(G * g, G * (g + 1))
        in_maps.append({
            "xt": np.ascontiguousarray(x[b].T).astype(np.float16),
            "wq": np.ascontiguousarray(Wq[hs, :].T).astype(np.float16),
            "wk": np.ascontiguousarray(Wk[hs, :].T).astype(np.float16),
            "wv": np.ascontiguousarray(Wv[hs, :].T).astype(np.float16),
            "wo": np.ascontiguousarray(Wo[:, hs].T).astype(np.float16),
            "mk": mk,
            "idn": idn,
        })

    if PROFILE:
        _install_profile_hooks()
    nc = _get_program()
    res = run_bass_kernel_spmd(nc, in_maps, core_ids=list(range(8)),
                               trace=PROFILE, tmpdir=PROFILE_DIR)
    LAST_RESULT = res
    parts = [res.results[c]["y"] for c in range(8)]
    out = np.stack([parts[2 * b] + parts[2 * b + 1] + bo for b in range(B)])
    return out.astype(np.float32)


# revision 11
# speedup vs baseline: 1.1501x; 1.1501x over previous
"""Causal multi-head attention block (B=4, S=2048, D=768, H=12, Dh=64)
distributed over 8 NeuronCores: core = (batch, head-group), each core
computes its 6 heads end-to-end plus its partial output projection;
host sums the two partials per batch and adds the bias.

Self-contained: hardcodes all shapes; no sibling imports.
"""

import numpy as np

B, S, D = 4, 2048, 768
H, DH = 12, 64
G = 384          # channels per head group (6 heads)
NPAIR = 3        # head pairs per core
NSC = 4          # 512-wide query windows
W = 512
NST = 16         # 128-row s-tiles
NDC = 6          # 128-row D chunks

_PROGRAM = None
PROFILE = False
PROFILE_DIR = None
LAST_RESULT = None


def _split_waits(nc, max_waits=1, max_updates=1):
    """This container's walrus rejects instructions carrying more than one
    semaphore wait/update ("Too many sync wait commands").  Move excess
    waits onto NoOps inserted before the owning instruction (same engine)
    and excess updates onto NoOps inserted after."""
    import concourse.mybir as mybir

    counter = [0]

    def nop(engine, waits, updates):
        counter[0] += 1
        n = mybir.InstNoOp(name=f"wsplit_nop_{counter[0]}", ins=[], outs=[])
        n.engine = engine
        n.sync_info = mybir.SyncInfo(on_wait=waits, on_update=updates)
        return n

    for bb in nc.main_func.blocks:
        out = []
        changed = False
        for ins in bb.instructions:
            si = ins.sync_info
            waits = list(si.on_wait) if si and si.on_wait else []
            updates = list(si.on_update) if si and si.on_update else []
            pre, post = [], []
            if len(waits) > max_waits:
                keep = waits[:max_waits - 1] if max_waits > 1 else []
                rest = waits[len(keep):]
                while rest:
                    chunk, rest = rest[:max_waits], rest[max_waits:]
                    pre.append(chunk)
                waits = keep
                changed = True
            if len(updates) > max_updates:
                rest = updates[max_updates:]
                updates = updates[:max_updates]
                while rest:
                    chunk, rest = rest[:max_updates], rest[max_updates:]
                    post.append(chunk)
                changed = True
            if pre or post:
                ins.sync_info = mybir.SyncInfo(
                    on_wait=waits, on_update=updates)
            for w in pre:
                out.append(nop(ins.engine, w, []))
            out.append(ins)
            for u in post:
                out.append(nop(ins.engine, [], u))
        if changed:
            bb.instructions = out


def _install_profile_hooks():
    """Dev-only (PROFILE=True): register the NTFF profile hook that the
    agent image's antenv lacks, and stub out the artifact upload."""
    import sys
    import types

    try:
        from antenv.axon_hooks import get_axon_ntff_profile_hook  # noqa: F401
    except ImportError:
        import antenv
        from trn_agent_boot import trn_boot

        hook = trn_boot._ntff_profile_via_ctypes("/opt/axon/libaxon_pjrt.so")
        mod = types.ModuleType("antenv.axon_hooks")
        mod._hook = hook
        mod.get_axon_ntff_profile_hook = lambda: mod._hook
        mod.set_axon_ntff_profile_hook = lambda h: setattr(mod, "_hook", h)
        sys.modules["antenv.axon_hooks"] = mod
        antenv.axon_hooks = mod

    from concourse import bass_utils

    bass_utils.upload_artifacts = lambda tmpdir: "local://" + tmpdir


def _build_program():
    import concourse.bass as bass
    import concourse.mybir as mybir
    import concourse.tile as tile

    f16 = mybir.dt.float16
    f32 = mybir.dt.float32

    nc = bass.Bass()
    xt_d = nc.declare_dram_parameter("xt", [D, S], f16, isOutput=False)
    wq_d = nc.declare_dram_parameter("wq", [D, G], f16, isOutput=False)
    wk_d = nc.declare_dram_parameter("wk", [D, G], f16, isOutput=False)
    wv_d = nc.declare_dram_parameter("wv", [D, G], f16, isOutput=False)
    wo_d = nc.declare_dram_parameter("wo", [G, D], f16, isOutput=False)
    mk_d = nc.declare_dram_parameter("mk", [128, 128], f16, isOutput=False)
    y_d = nc.declare_dram_parameter("y", [S, D], f32, isOutput=True)

    with tile.TileContext(nc) as tc:
        with (
            tc.tile_pool(name="const", bufs=1) as const,
            tc.tile_pool(name="work", bufs=3) as work,
            tc.tile_pool(name="outp", bufs=3) as outp,
            tc.tile_pool(name="ps", bufs=2, space="PSUM") as ps,
        ):
            # ---- persistent SBUF tiles ----
            xt = [const.tile([128, S], f16, name=f"xt{i}", tag=f"xt{i}")
                  for i in range(NDC)]
            wq = [const.tile([128, G], f16, name=f"wq{i}", tag=f"wq{i}")
                  for i in range(NDC)]
            wk = [const.tile([128, G], f16, name=f"wk{i}", tag=f"wk{i}")
                  for i in range(NDC)]
            wv = [const.tile([128, G], f16, name=f"wv{i}", tag=f"wv{i}")
                  for i in range(NDC)]
            wo = [const.tile([128, D], f16, name=f"wo{i}", tag=f"wo{i}")
                  for i in range(3)]
            qt = [const.tile([128, S], f16, name=f"qt{p}", tag=f"qt{p}")
                  for p in range(NPAIR)]
            kt = [const.tile([128, S], f16, name=f"kt{p}", tag=f"kt{p}")
                  for p in range(NPAIR)]
            vt = [const.tile([128, G], f16, name=f"vt{t}", tag=f"vt{t}")
                  for t in range(NST)]
            gt = [const.tile([128, S], f16, name=f"gt{p}", tag=f"gt{p}")
                  for p in range(NPAIR)]
            mk = const.tile([128, 128], f16, name="mk", tag="mk")
            ones = const.tile([128, DH], f16, name="ones", tag="ones")

            # ---- input DMAs (weights first so the first projection
            # group can start as soon as xt chunk 0 lands) ----
            nc.sync.dma_start(out=mk, in_=mk_d[:, :])
            for i in range(NDC):
                nc.sync.dma_start(out=wq[i], in_=wq_d[128 * i:128 * (i + 1), :])
                nc.sync.dma_start(out=wk[i], in_=wk_d[128 * i:128 * (i + 1), :])
                nc.sync.dma_start(out=wv[i], in_=wv_d[128 * i:128 * (i + 1), :])
            for i in range(NDC):
                nc.sync.dma_start(out=xt[i], in_=xt_d[128 * i:128 * (i + 1), :])
            for i in range(3):
                nc.sync.dma_start(out=wo[i], in_=wo_d[128 * i:128 * (i + 1), :])
            nc.vector.memset(ones, 1.0)

            # Pre-clear the two scores slots so diagonal-group exp calls that
            # sweep across the unwritten gap read defined values.
            for i in range(3):
                z = ps.tile([128, 1024], f32, name=f"z{i}", tag="sc", bufs=3)
                nc.vector.memset(z, 0.0)

            def proj_qk_unit(pair, sc):
                qp = ps.tile([128, W], f32, name=f"qp{pair}_{sc}",
                             tag="apv", bufs=1)
                for dc in range(NDC):
                    nc.tensor.matmul(
                        qp,
                        wq[dc][:, 128 * pair:128 * (pair + 1)],
                        xt[dc][:, W * sc:W * (sc + 1)],
                        start=(dc == 0), stop=(dc == NDC - 1))
                nc.vector.tensor_copy(
                    out=qt[pair][:, W * sc:W * (sc + 1)], in_=qp)
                kp = ps.tile([128, W], f32, name=f"kp{pair}_{sc}",
                             tag="adn", bufs=1)
                for dc in range(NDC):
                    nc.tensor.matmul(
                        kp,
                        wk[dc][:, 128 * pair:128 * (pair + 1)],
                        xt[dc][:, W * sc:W * (sc + 1)],
                        start=(dc == 0), stop=(dc == NDC - 1))
                nc.vector.tensor_copy(
                    out=kt[pair][:, W * sc:W * (sc + 1)], in_=kp)

            def proj_v(st):
                vp = ps.tile([128, G], f32, name=f"vp{st}", tag="apv", bufs=1)
                for dc in range(NDC):
                    nc.tensor.matmul(
                        vp,
                        xt[dc][:, 128 * st:128 * (st + 1)],
                        wv[dc],
                        start=(dc == 0), stop=(dc == NDC - 1))
                nc.vector.tensor_copy(out=vt[st], in_=vp)

            def outproj(st):
                o0 = ps.tile([128, G], f32, name=f"o0_{st}", tag="apv", bufs=1)
                for cc in range(3):
                    nc.tensor.matmul(
                        o0,
                        gt[cc][:, 128 * st:128 * (st + 1)],
                        wo[cc][:, 0:G],
                        start=(cc == 0), stop=(cc == 2))
                o1 = ps.tile([128, G], f32, name=f"o1_{st}", tag="adn", bufs=1)
                for cc in range(3):
                    nc.tensor.matmul(
                        o1,
                        gt[cc][:, 128 * st:128 * (st + 1)],
                        wo[cc][:, G:D],
                        start=(cc == 0), stop=(cc == 2))
                ob = outp.tile([128, D], f32, name=f"ob{st}", tag="ob", bufs=3)
                nc.vector.tensor_copy(out=ob[:, 0:G], in_=o0)
                nc.vector.tensor_copy(out=ob[:, G:D], in_=o1)
                nc.sync.dma_start(
                    out=y_d[128 * st:128 * (st + 1), :], in_=ob)

            def attention(pair, sc, filler_tick):
                jbs = list(range(4 * sc + 4))
                pv = ps.tile([128, W], f32, name=f"pv{pair}_{sc}",
                             tag="apv", bufs=1)
                dn = ps.tile([128, W], f32, name=f"dn{pair}_{sc}",
                             tag="adn", bufs=1)
                for jb in jbs:
                    col0 = max(0, 128 * jb - W * sc)
                    first, last = (jb == jbs[0]), (jb == jbs[-1])
                    diag = jb >= 4 * sc
                    sct = ps.tile([128, 1024], f32, name=f"sc{pair}_{sc}_{jb}",
                                  tag="sc", bufs=3)
                    # scores^T for the head pair (row-packed: K=64 each)
                    nc.tensor.matmul(
                        sct[:, col0:W],
                        kt[pair][0:64, 128 * jb:128 * (jb + 1)],
                        qt[pair][0:64, W * sc + col0:W * (sc + 1)],
                        start=True, stop=True)
                    nc.tensor.matmul(
                        sct[:, W + col0:2 * W],
                        kt[pair][64:128, 128 * jb:128 * (jb + 1)],
                        qt[pair][64:128, W * sc + col0:W * (sc + 1)],
                        start=True, stop=True)
                    ex = work.tile([128, 1024], f16, name=f"ex{pair}_{sc}_{jb}",
                                   tag="exp", bufs=4)
                    nc.scalar.activation(
                        out=ex[:, col0:1024], in_=sct[:, col0:1024],
                        func=mybir.ActivationFunctionType.Exp, scale=0.125)
                    if diag:  # zero the j>i triangle of the diagonal block
                        nc.vector.tensor_mul(
                            ex[:, col0:col0 + 128], ex[:, col0:col0 + 128], mk)
                        nc.vector.tensor_mul(
                            ex[:, W + col0:W + col0 + 128],
                            ex[:, W + col0:W + col0 + 128], mk)
                    # attn @ v (transposed out, col-packed pair) + denominators
                    nc.tensor.matmul(
                        pv[0:64, col0:W],
                        vt[jb][:, 128 * pair:128 * pair + 64],
                        ex[:, col0:W],
                        start=first, stop=last)
                    nc.tensor.matmul(
                        pv[64:128, col0:W],
                        vt[jb][:, 128 * pair + 64:128 * (pair + 1)],
                        ex[:, W + col0:2 * W],
                        start=first, stop=last)
                    nc.tensor.matmul(
                        dn[0:64, col0:W],
                        ones,
                        ex[:, col0:W],
                        start=first, stop=last)
                    nc.tensor.matmul(
                        dn[64:128, col0:W],
                        ones,
                        ex[:, W + col0:2 * W],
                        start=first, stop=last)
                    filler_tick()
                rc = work.tile([128, W], f32, name=f"rc{pair}_{sc}",
                               tag="rc", bufs=2)
                nc.vector.reciprocal(out=rc, in_=dn)
                nc.vector.tensor_mul(
                    gt[pair][:, W * sc:W * (sc + 1)], pv, rc)

            # ---- orchestration: coarse phase interleave ----
            for sc in range(NSC):
                proj_qk_unit(0, sc)
            for st in range(4):
                proj_v(st)
            for sc in range(NSC):
                if sc + 1 < NSC:
                    for st in range(4 * (sc + 1), 4 * (sc + 2)):
                        proj_v(st)
                attention(0, sc, lambda: None)
            for sc in range(NSC):
                proj_qk_unit(1, sc)
            for sc in range(NSC):
                attention(1, sc, lambda: None)
            for sc in range(NSC):
                proj_qk_unit(2, sc)
            for sc in range(NSC):
                attention(2, sc, lambda: None)
            for st in range(NST):
                outproj(st)

    _split_waits(nc)
    return nc


def _get_program():
    global _PROGRAM
    if _PROGRAM is None:
        _PROGRAM = _build_program()
    return _PROGRAM


def kernel(x, Wq, Wk, Wv, Wo, bo):
    global LAST_RESULT
    from concourse.bass_utils import run_bass_kernel_spmd

    x = np.asarray(x, np.float32)
    Wq = np.asarray(Wq, np.float32)
    Wk = np.asarray(Wk, np.float32)
    Wv = np.asarray(Wv, np.float32)
    Wo = np.asarray(Wo, np.float32)
    bo = np.asarray(bo, np.float32)

    tri = np.tril(np.ones((128, 128), np.float32)).T  # 1 where j<=i
    mk = tri.astype(np.float16)

    in_maps = []
    for c in range(8):
        b, g = divmod(c, 2)
        hs = slice(G * g, G * (g + 1))
        in_maps.append({
            "xt": np.ascontiguousarray(x[b].T).astype(np.float16),
            "wq": np.ascontiguousarray(Wq[hs, :].T).astype(np.float16),
            "wk": np.ascontiguousarray(Wk[hs, :].T).astype(np.float16),
            "wv": np.ascontiguousarray(Wv[hs, :].T).astype(np.float16),
            "wo": np.ascontiguousarray(Wo[:, hs].T).astype(np.float16),
            "mk": mk,
        })

    if PROFILE:
        _install_profile_hooks()
    nc = _get_program()
    res = run_bass_kernel_spmd(nc, in_maps, core_ids=list(range(8)),
                               trace=PROFILE, tmpdir=PROFILE_DIR)
    LAST_RESULT = res
    parts = [res.results[c]["y"] for c in range(8)]
    out = np.stack([parts[2 * b] + parts[2 * b + 1] + bo for b in range(B)])
    return out.astype(np.float32)


# revision 12
# speedup vs baseline: 1.1660x; 1.0138x over previous
"""Causal multi-head attention block (B=4, S=2048, D=768, H=12, Dh=64)
distributed over 8 NeuronCores: core = (batch, head-group), each core
computes its 6 heads end-to-end plus its partial output projection;
host sums the two partials per batch and adds the bias.

Self-contained: hardcodes all shapes; no sibling imports.
"""

import numpy as np

B, S, D = 4, 2048, 768
H, DH = 12, 64
G = 384          # channels per head group (6 heads)
NPAIR = 3        # head pairs per core
NSC = 4          # 512-wide query windows
W = 512
NST = 16         # 128-row s-tiles
NDC = 6          # 128-row D chunks

_PROGRAM = None
PROFILE = False
PROFILE_DIR = None
LAST_RESULT = None


def _split_waits(nc, max_waits=1, max_updates=1):
    """This container's walrus rejects instructions carrying more than one
    semaphore wait/update ("Too many sync wait commands").  Move excess
    waits onto NoOps inserted before the owning instruction (same engine)
    and excess updates onto NoOps inserted after."""
    import concourse.mybir as mybir

    counter = [0]

    def nop(engine, waits, updates):
        counter[0] += 1
        n = mybir.InstNoOp(name=f"wsplit_nop_{counter[0]}", ins=[], outs=[])
        n.engine = engine
        n.sync_info = mybir.SyncInfo(on_wait=waits, on_update=updates)
        return n

    for bb in nc.main_func.blocks:
        out = []
        changed = False
        for ins in bb.instructions:
            si = ins.sync_info
            waits = list(si.on_wait) if si and si.on_wait else []
            updates = list(si.on_update) if si and si.on_update else []
            pre, post = [], []
            if len(waits) > max_waits:
                keep = waits[:max_waits - 1] if max_waits > 1 else []
                rest = waits[len(keep):]
                while rest:
                    chunk, rest = rest[:max_waits], rest[max_waits:]
                    pre.append(chunk)
                waits = keep
                changed = True
            if len(updates) > max_updates:
                rest = updates[max_updates:]
                updates = updates[:max_updates]
                while rest:
                    chunk, rest = rest[:max_updates], rest[max_updates:]
                    post.append(chunk)
                changed = True
            if pre or post:
                ins.sync_info = mybir.SyncInfo(
                    on_wait=waits, on_update=updates)
            for w in pre:
                out.append(nop(ins.engine, w, []))
            out.append(ins)
            for u in post:
                out.append(nop(ins.engine, [], u))
        if changed:
            bb.instructions = out


def _install_profile_hooks():
    """Dev-only (PROFILE=True): register the NTFF profile hook that the
    agent image's antenv lacks, and stub out the artifact upload."""
    import sys
    import types

    try:
        from antenv.axon_hooks import get_axon_ntff_profile_hook  # noqa: F401
    except ImportError:
        import antenv
        from trn_agent_boot import trn_boot

        hook = trn_boot._ntff_profile_via_ctypes("/opt/axon/libaxon_pjrt.so")
        mod = types.ModuleType("antenv.axon_hooks")
        mod._hook = hook
        mod.get_axon_ntff_profile_hook = lambda: mod._hook
        mod.set_axon_ntff_profile_hook = lambda h: setattr(mod, "_hook", h)
        sys.modules["antenv.axon_hooks"] = mod
        antenv.axon_hooks = mod

    from concourse import bass_utils

    bass_utils.upload_artifacts = lambda tmpdir: "local://" + tmpdir


def _build_program():
    import concourse.bass as bass
    import concourse.mybir as mybir
    import concourse.tile as tile

    f16 = mybir.dt.float16
    f32 = mybir.dt.float32

    nc = bass.Bass()
    xt_d = nc.declare_dram_parameter("xt", [D, S], f16, isOutput=False)
    wq_d = nc.declare_dram_parameter("wq", [D, G], f16, isOutput=False)
    wk_d = nc.declare_dram_parameter("wk", [D, G], f16, isOutput=False)
    wv_d = nc.declare_dram_parameter("wv", [D, G], f16, isOutput=False)
    wo_d = nc.declare_dram_parameter("wo", [G, D], f16, isOutput=False)
    mk_d = nc.declare_dram_parameter("mk", [128, 128], f16, isOutput=False)
    y_d = nc.declare_dram_parameter("y", [S, D], f32, isOutput=True)

    with tile.TileContext(nc) as tc:
        with (
            tc.tile_pool(name="const", bufs=1) as const,
            tc.tile_pool(name="work", bufs=3) as work,
            tc.tile_pool(name="outp", bufs=3) as outp,
            tc.tile_pool(name="ps", bufs=2, space="PSUM") as ps,
        ):
            # ---- persistent SBUF tiles ----
            xt = [const.tile([128, S], f16, name=f"xt{i}", tag=f"xt{i}")
                  for i in range(NDC)]
            wq = [const.tile([128, G], f16, name=f"wq{i}", tag=f"wq{i}")
                  for i in range(NDC)]
            wk = [const.tile([128, G], f16, name=f"wk{i}", tag=f"wk{i}")
                  for i in range(NDC)]
            wv = [const.tile([128, G], f16, name=f"wv{i}", tag=f"wv{i}")
                  for i in range(NDC)]
            wo = [const.tile([128, D], f16, name=f"wo{i}", tag=f"wo{i}")
                  for i in range(3)]
            qt = [const.tile([128, S], f16, name=f"qt{p}", tag=f"qt{p}")
                  for p in range(NPAIR)]
            kt = [const.tile([128, S], f16, name=f"kt{p}", tag=f"kt{p}")
                  for p in range(NPAIR)]
            vt = [const.tile([128, G], f16, name=f"vt{t}", tag=f"vt{t}")
                  for t in range(NST)]
            gt = [const.tile([128, S], f16, name=f"gt{p}", tag=f"gt{p}")
                  for p in range(NPAIR)]
            mk = const.tile([128, 128], f16, name="mk", tag="mk")
            ones = const.tile([128, DH], f16, name="ones", tag="ones")

            # ---- input DMAs (weights first so the first projection
            # group can start as soon as xt chunk 0 lands) ----
            nc.sync.dma_start(out=mk, in_=mk_d[:, :])
            for i in range(NDC):
                nc.sync.dma_start(out=wq[i], in_=wq_d[128 * i:128 * (i + 1), :])
                nc.sync.dma_start(out=wk[i], in_=wk_d[128 * i:128 * (i + 1), :])
                nc.sync.dma_start(out=wv[i], in_=wv_d[128 * i:128 * (i + 1), :])
            for i in range(NDC):
                nc.sync.dma_start(out=xt[i], in_=xt_d[128 * i:128 * (i + 1), :])
            for i in range(3):
                nc.sync.dma_start(out=wo[i], in_=wo_d[128 * i:128 * (i + 1), :])
            nc.vector.memset(ones, 1.0)

            # Pre-clear the two scores slots so diagonal-group exp calls that
            # sweep across the unwritten gap read defined values.
            for i in range(3):
                z = ps.tile([128, 1024], f32, name=f"z{i}", tag="sc", bufs=3)
                nc.vector.memset(z, 0.0)

            def act_recip(out, in_):
                # ScalarE table reciprocal (~1e-5 rel err on [1e-2, 1e7],
                # verified on HW) -- keeps the softmax divide off the DVE
                # and off the inter-window critical path.
                eng = nc.scalar
                ins_ = [eng.lower_ap(in_[:, :]),
                        mybir.ImmediateValue(dtype=mybir.dt.float32, value=0.0),
                        mybir.ImmediateValue(dtype=mybir.dt.float32, value=1.0),
                        mybir.ImmediateValue(dtype=mybir.dt.float32, value=0.0)]
                eng.add_instruction(mybir.InstActivation(
                    name=nc.get_next_instruction_name(),
                    func=mybir.ActivationFunctionType.Reciprocal,
                    ins=ins_, outs=[eng.lower_ap(out[:, :])]))

            def proj_qk_unit(pair, sc):
                qp = ps.tile([128, W], f32, name=f"qp{pair}_{sc}",
                             tag="apv", bufs=1)
                for dc in range(NDC):
                    nc.tensor.matmul(
                        qp,
                        wq[dc][:, 128 * pair:128 * (pair + 1)],
                        xt[dc][:, W * sc:W * (sc + 1)],
                        start=(dc == 0), stop=(dc == NDC - 1))
                nc.vector.tensor_copy(
                    out=qt[pair][:, W * sc:W * (sc + 1)], in_=qp)
                kp = ps.tile([128, W], f32, name=f"kp{pair}_{sc}",
                             tag="adn", bufs=1)
                for dc in range(NDC):
                    nc.tensor.matmul(
                        kp,
                        wk[dc][:, 128 * pair:128 * (pair + 1)],
                        xt[dc][:, W * sc:W * (sc + 1)],
                        start=(dc == 0), stop=(dc == NDC - 1))
                nc.vector.tensor_copy(
                    out=kt[pair][:, W * sc:W * (sc + 1)], in_=kp)

            def proj_v(st):
                vp = ps.tile([128, G], f32, name=f"vp{st}", tag="apv", bufs=1)
                for dc in range(NDC):
                    nc.tensor.matmul(
                        vp,
                        xt[dc][:, 128 * st:128 * (st + 1)],
                        wv[dc],
                        start=(dc == 0), stop=(dc == NDC - 1))
                nc.vector.tensor_copy(out=vt[st], in_=vp)

            def outproj(st):
                o0 = ps.tile([128, G], f32, name=f"o0_{st}", tag="apv", bufs=1)
                for cc in range(3):
                    nc.tensor.matmul(
                        o0,
                        gt[cc][:, 128 * st:128 * (st + 1)],
                        wo[cc][:, 0:G],
                        start=(cc == 0), stop=(cc == 2))
                o1 = ps.tile([128, G], f32, name=f"o1_{st}", tag="adn", bufs=1)
                for cc in range(3):
                    nc.tensor.matmul(
                        o1,
                        gt[cc][:, 128 * st:128 * (st + 1)],
                        wo[cc][:, G:D],
                        start=(cc == 0), stop=(cc == 2))
                ob = outp.tile([128, D], f32, name=f"ob{st}", tag="ob", bufs=3)
                nc.vector.tensor_copy(out=ob[:, 0:G], in_=o0)
                nc.vector.tensor_copy(out=ob[:, G:D], in_=o1)
                nc.sync.dma_start(
                    out=y_d[128 * st:128 * (st + 1), :], in_=ob)

            def attention(pair, sc, filler_tick):
                jbs = list(range(4 * sc + 4))
                pv = ps.tile([128, W], f32, name=f"pv{pair}_{sc}",
                             tag="apv", bufs=1)
                dn = ps.tile([128, W], f32, name=f"dn{pair}_{sc}",
                             tag="adn", bufs=1)
                for jb in jbs:
                    col0 = max(0, 128 * jb - W * sc)
                    first, last = (jb == jbs[0]), (jb == jbs[-1])
                    diag = jb >= 4 * sc
                    sct = ps.tile([128, 1024], f32, name=f"sc{pair}_{sc}_{jb}",
                                  tag="sc", bufs=3)
                    # scores^T for the head pair (row-packed: K=64 each)
                    nc.tensor.matmul(
                        sct[:, col0:W],
                        kt[pair][0:64, 128 * jb:128 * (jb + 1)],
                        qt[pair][0:64, W * sc + col0:W * (sc + 1)],
                        start=True, stop=True)
                    nc.tensor.matmul(
                        sct[:, W + col0:2 * W],
                        kt[pair][64:128, 128 * jb:128 * (jb + 1)],
                        qt[pair][64:128, W * sc + col0:W * (sc + 1)],
                        start=True, stop=True)
                    ex = work.tile([128, 1024], f16, name=f"ex{pair}_{sc}_{jb}",
                                   tag="exp", bufs=4)
                    nc.scalar.activation(
                        out=ex[:, col0:1024], in_=sct[:, col0:1024],
                        func=mybir.ActivationFunctionType.Exp, scale=0.125)
                    if diag:  # zero the j>i triangle of the diagonal block
                        nc.vector.tensor_mul(
                            ex[:, col0:col0 + 128], ex[:, col0:col0 + 128], mk)
                        nc.vector.tensor_mul(
                            ex[:, W + col0:W + col0 + 128],
                            ex[:, W + col0:W + col0 + 128], mk)
                    # attn @ v (transposed out, col-packed pair) + denominators
                    nc.tensor.matmul(
                        pv[0:64, col0:W],
                        vt[jb][:, 128 * pair:128 * pair + 64],
                        ex[:, col0:W],
                        start=first, stop=last)
                    nc.tensor.matmul(
                        pv[64:128, col0:W],
                        vt[jb][:, 128 * pair + 64:128 * (pair + 1)],
                        ex[:, W + col0:2 * W],
                        start=first, stop=last)
                    nc.tensor.matmul(
                        dn[0:64, col0:W],
                        ones,
                        ex[:, col0:W],
                        start=first, stop=last)
                    nc.tensor.matmul(
                        dn[64:128, col0:W],
                        ones,
                        ex[:, W + col0:2 * W],
                        start=first, stop=last)
                    filler_tick()
                rc = work.tile([128, W], f32, name=f"rc{pair}_{sc}",
                               tag="rc", bufs=2)
                act_recip(rc, dn)
                nc.vector.tensor_mul(
                    gt[pair][:, W * sc:W * (sc + 1)], pv, rc)

            # ---- orchestration: coarse phase interleave ----
            for sc in range(NSC):
                proj_qk_unit(0, sc)
            for st in range(4):
                proj_v(st)
            for sc in range(NSC):
                if sc + 1 < NSC:
                    for st in range(4 * (sc + 1), 4 * (sc + 2)):
                        proj_v(st)
                attention(0, sc, lambda: None)
            for sc in range(NSC):
                proj_qk_unit(1, sc)
            for sc in range(NSC):
                attention(1, sc, lambda: None)
            for sc in range(NSC):
                proj_qk_unit(2, sc)
            for sc in range(NSC):
                attention(2, sc, lambda: None)
            for st in range(NST):
                outproj(st)

    _split_waits(nc)
    return nc


def _get_program():
    global _PROGRAM
    if _PROGRAM is None:
        _PROGRAM = _build_program()
    return _PROGRAM


def kernel(x, Wq, Wk, Wv, Wo, bo):
    global LAST_RESULT
    from concourse.bass_utils import run_bass_kernel_spmd

    x = np.asarray(x, np.float32)
    Wq = np.asarray(Wq, np.float32)
    Wk = np.asarray(Wk, np.float32)
    Wv = np.asarray(Wv, np.float32)
    Wo = np.asarray(Wo, np.float32)
    bo = np.asarray(bo, np.float32)

    tri = np.tril(np.ones((128, 128), np.float32)).T  # 1 where j<=i
    mk = tri.astype(np.float16)

    in_maps = []
    for c in range(8):
        b, g = divmod(c, 2)
        hs = slice(G * g, G * (g + 1))
        in_maps.append({
            "xt": np.ascontiguousarray(x[b].T).astype(np.float16),
            "wq": np.ascontiguousarray(Wq[hs, :].T).astype(np.float16),
            "wk": np.ascontiguousarray(Wk[hs, :].T).astype(np.float16),
            "wv": np.ascontiguousarray(Wv[hs, :].T).astype(np.float16),
            "wo": np.ascontiguousarray(Wo[:, hs].T).astype(np.float16),
            "mk": mk,
        })

    if PROFILE:
        _install_profile_hooks()
    nc = _get_program()
    res = run_bass_kernel_spmd(nc, in_maps, core_ids=list(range(8)),
                               trace=PROFILE, tmpdir=PROFILE_DIR)
    LAST_RESULT = res
    parts = [res.results[c]["y"] for c in range(8)]
    out = np.stack([parts[2 * b] + parts[2 * b + 1] + bo for b in range(B)])
    return out.astype(np.float32)
